# revision 1
# baseline (speedup 1.0000x reference)
"""Trainium2 Bass kernel for nn_NodeModel (GNN message passing).

Reference computation:
    agg = segment_sum(edge_attr, edge_index[1], num_segments=N)     # scatter-add
    h   = relu(concat([x, agg, u[batch]], 1) @ W1 + b1)
    out = h @ W2 + b2 + x

Strategy (8 NeuronCores, graph-parallel by destination node):
  - Nodes are padded to 100352 = 8 * 12544 and sharded contiguously across 8
    cores. Each core owns 12544 destination nodes = 49 ranges of 256 nodes.
  - Host groups edges by destination range (counting-sort), pads each range's
    edge list to a multiple of 128, and lays the per-core edge features out
    contiguously. Rebased destination columns (col % 256, pad = -1) ride along.
  - On device, the scatter-add is computed on the TensorEngine as a sequence of
    one-hot matmuls: for each 128-edge block, DVE builds a one-hot [128e, 256n]
    via tensor_scalar(is_equal) against an iota row; PE accumulates
    edge_blockT @ onehot into a PSUM bank per 256-node range, giving aggT
    [128 feat, 256 nodes] directly (no transposes needed downstream).
  - MLP runs per 512-node group: h1T[h,n] = W1x.T xT + W1a.T aggT + W1u.T ugT
    accumulated in PSUM, ReLU+bias on ScalarE during evacuation; layer 2
    produces natural-orientation out[n,d] with the residual (+x) folded in as
    an identity matmul from xT and the bias as a rank-1 matmul.
  - All matmul operands use float32r (TF32-like, 4-byte) — no casts needed.
"""

import os
from contextlib import ExitStack

import ml_dtypes
import numpy as np

N_NODES = 100000
N_EDGES = 1600000
D = 128          # node / edge feature dim
DG = 16          # global feature dim
H = 256          # hidden dim
G = 64           # graphs
NCORES = 8

NPC = 12544      # nodes per core (= 98 * 128 = 49 * 256)
N_PAD = NCORES * NPC
RW = 128         # scatter range width (nodes per PSUM accumulation group)
RPC = NPC // RW  # 98 ranges per core
EBLK = 128       # edges per matmul block
CHUNK_BLKS = 32  # edge blocks per DMA chunk (4096 edges = 2 MiB)

NB_MLP = 512     # nodes per MLP group

_PROFILE_RESULTS = [None]  # stash for test harness introspection


def _shard_inputs(x, edge_index, edge_attr, u, batch, W1, b1, W2, b2):
    x = np.ascontiguousarray(np.asarray(x, dtype=np.float32))
    edge_index = np.asarray(edge_index)
    edge_attr = np.ascontiguousarray(np.asarray(edge_attr, dtype=np.float32))
    u = np.asarray(u, dtype=np.float32)
    batch = np.asarray(batch)
    W1 = np.asarray(W1, dtype=np.float32)
    b1 = np.asarray(b1, dtype=np.float32)
    W2 = np.asarray(W2, dtype=np.float32)
    b2 = np.asarray(b2, dtype=np.float32)

    col = np.asarray(edge_index[1], dtype=np.int64)
    r_glob = (col // RW).astype(np.int64)           # global 256-node range id
    n_ranges = NCORES * RPC

    counts = np.bincount(r_glob, minlength=n_ranges)
    cnt_cl = counts.reshape(NCORES, RPC)
    # blocks per local range: shared across cores (same SPMD program)
    B = np.maximum(1, (cnt_cl.max(axis=0) + EBLK - 1) // EBLK).astype(np.int64)
    prefix = np.concatenate([[0], np.cumsum(B)])    # [RPC+1]
    nblk = int(prefix[-1])                          # blocks per core
    s_slots = nblk * EBLK
    nchunk = (s_slots + CHUNK_BLKS * EBLK - 1) // (CHUNK_BLKS * EBLK)
    s_alloc = nchunk * CHUNK_BLKS * EBLK
    nblk_alloc = s_alloc // EBLK

    order = np.argsort(r_glob, kind="stable")
    sorted_r = r_glob[order]
    starts = np.concatenate([[0], np.cumsum(counts)])[:-1]
    rank = np.arange(N_EDGES, dtype=np.int64) - starts[sorted_r]
    l_of = sorted_r % RPC
    core_of = sorted_r // RPC
    dst_slot = prefix[l_of] * EBLK + rank

    # swizzled edge layout: [core, chunk, p, blk_in_chunk, feat] so each chunk's
    # DMA is a fully contiguous [128, CHUNK_BLKS*128] 2D slice per partition
    blk_of = dst_slot // EBLK
    ea_all = np.zeros((NCORES, nchunk, EBLK, CHUNK_BLKS, D), dtype=np.float32)
    ea_all[core_of, blk_of // CHUNK_BLKS, dst_slot % EBLK, blk_of % CHUNK_BLKS] = (
        edge_attr[order]
    )
    ea_all = ea_all.reshape(NCORES, nchunk * EBLK, CHUNK_BLKS * D)
    colr_all = np.full((NCORES, s_alloc), -1.0, dtype=np.float32)
    colr_all[core_of, dst_slot] = (col[order] % RW).astype(np.float32)
    # [core, 128, nblk_alloc]: colrT[c, p, blk] = rebased col of edge slot blk*128+p
    colrT_all = np.ascontiguousarray(
        colr_all.reshape(NCORES, nblk_alloc, EBLK).transpose(0, 2, 1)
    )

    x_pad = np.zeros((N_PAD, D), dtype=np.float32)
    x_pad[:N_NODES] = x
    xT_all = np.ascontiguousarray(x_pad.reshape(NCORES, NPC, D).transpose(0, 2, 1))

    batch_pad = np.concatenate(
        [batch, np.full(N_PAD - N_NODES, batch[-1], dtype=batch.dtype)]
    ).astype(np.int64)
    ug = u[batch_pad]                                # [N_PAD, DG]
    ugT_all = np.ascontiguousarray(ug.reshape(NCORES, NPC, DG).transpose(0, 2, 1))

    consts = {
        "w1x": np.ascontiguousarray(W1[:D]),                  # [128, 256]
        "w1a": np.ascontiguousarray(W1[D : 2 * D]),           # [128, 256]
        "w1u": np.ascontiguousarray(W1[2 * D :]),             # [16, 256]
        "b1t": np.ascontiguousarray(b1.reshape(2, D).T),      # [128, 2]
        "w2a": np.ascontiguousarray(W2[:D]),                  # [128, 128]
        "w2b": np.ascontiguousarray(W2[D:]),                  # [128, 128]
        "b2r": np.ascontiguousarray(b2[None, :]),             # [1, 128]
        "ones": np.ones((1, NB_MLP), dtype=np.float32),
        "ident": np.eye(D, dtype=np.float32),
        "iota": np.tile(np.arange(RW, dtype=np.float32), (D, 1)).astype(ml_dtypes.bfloat16),
    }

    in_maps = []
    for c in range(NCORES):
        m = {
            "ea": ea_all[c],
            "colrt": colrT_all[c],
            "xt": xT_all[c],
            "ugt": ugT_all[c],
        }
        m.update(consts)
        in_maps.append(m)
    return in_maps, B, nchunk, nblk_alloc


def _build_program(B, nchunk, nblk_alloc, loop_n=1, variant="full"):
    import concourse.bacc as bacc
    import concourse.mybir as mybir
    import concourse.tile as tile

    F32 = mybir.dt.float32
    F32R = mybir.dt.float32r
    BF16 = mybir.dt.bfloat16
    s_alloc = nchunk * CHUNK_BLKS * EBLK
    prefix = np.concatenate([[0], np.cumsum(B)])

    nc = bacc.Bacc("TRN2", target_bir_lowering=False, debug=False)

    ea_d = nc.dram_tensor("ea", [nchunk * EBLK, CHUNK_BLKS * D], F32,
                          kind="ExternalInput")
    colrt_d = nc.dram_tensor("colrt", [EBLK, nblk_alloc], F32, kind="ExternalInput")
    xt_d = nc.dram_tensor("xt", [D, NPC], F32R, kind="ExternalInput")
    ugt_d = nc.dram_tensor("ugt", [DG, NPC], F32R, kind="ExternalInput")
    w1x_d = nc.dram_tensor("w1x", [D, H], F32R, kind="ExternalInput")
    w1a_d = nc.dram_tensor("w1a", [D, H], F32R, kind="ExternalInput")
    w1u_d = nc.dram_tensor("w1u", [DG, H], F32R, kind="ExternalInput")
    b1t_d = nc.dram_tensor("b1t", [D, 2], F32, kind="ExternalInput")
    w2a_d = nc.dram_tensor("w2a", [D, D], F32R, kind="ExternalInput")
    w2b_d = nc.dram_tensor("w2b", [D, D], F32R, kind="ExternalInput")
    b2r_d = nc.dram_tensor("b2r", [1, D], F32R, kind="ExternalInput")
    ones_d = nc.dram_tensor("ones", [1, NB_MLP], F32R, kind="ExternalInput")
    ident_d = nc.dram_tensor("ident", [D, D], F32R, kind="ExternalInput")
    iota_d = nc.dram_tensor("iota", [D, RW], BF16, kind="ExternalInput")
    out_d = nc.dram_tensor("out", [NPC, D], F32, kind="ExternalOutput")

    import contextlib

    with tile.TileContext(nc) as tc, ExitStack() as ctx:
        persist = ctx.enter_context(tc.tile_pool(name="persist", bufs=1))
        ea_pool = ctx.enter_context(tc.tile_pool(name="ea", bufs=3))
        eabf_pool = ctx.enter_context(tc.tile_pool(name="eabf", bufs=3))
        oh_pool = ctx.enter_context(tc.tile_pool(name="oh", bufs=24))
        agg_pool = ctx.enter_context(tc.tile_pool(name="agg", bufs=8))
        ug_pool = ctx.enter_context(tc.tile_pool(name="ug", bufs=2))
        hs_pool = ctx.enter_context(tc.tile_pool(name="hs", bufs=4))
        os_pool = ctx.enter_context(tc.tile_pool(name="os", bufs=2))
        o2sb_pool = ctx.enter_context(tc.tile_pool(name="o2sb", bufs=2))
        sc_psum = ctx.enter_context(tc.tile_pool(name="scps", bufs=2, space="PSUM"))
        h_psum = ctx.enter_context(tc.tile_pool(name="hps", bufs=2, space="PSUM"))
        o2_psum = ctx.enter_context(tc.tile_pool(name="o2ps", bufs=2, space="PSUM"))
        t_psum = ctx.enter_context(tc.tile_pool(name="tps", bufs=2, space="PSUM"))

        # --- persistent loads -------------------------------------------------
        def pload(dram, shape, dtype, engine):
            t = persist.tile(shape, dtype, tag=dram.name)
            engine.dma_start(t[:], dram.ap())
            return t

        w1x_t = pload(w1x_d, [D, H], F32R, nc.scalar)
        w1a_t = pload(w1a_d, [D, H], F32R, nc.scalar)
        w1u_t = pload(w1u_d, [DG, H], F32R, nc.scalar)
        b1t_t = pload(b1t_d, [D, 2], F32, nc.scalar)
        w2a_t = pload(w2a_d, [D, D], F32R, nc.scalar)
        w2b_t = pload(w2b_d, [D, D], F32R, nc.scalar)
        b2r_t = pload(b2r_d, [1, D], F32R, nc.scalar)
        ones_t = pload(ones_d, [1, NB_MLP], F32R, nc.scalar)
        ident_t = pload(ident_d, [D, D], F32R, nc.scalar)
        iota_t = pload(iota_d, [D, RW], BF16, nc.scalar)
        colrt_t = pload(colrt_d, [EBLK, nblk_alloc], F32, nc.scalar)
        xt_t = pload(xt_d, [D, NPC], F32R, nc.scalar)

        chunk_tiles = {}

        def get_chunk(ci):
            if ci not in chunk_tiles:
                if variant == "dmacast":
                    t = eabf_pool.tile([EBLK, CHUNK_BLKS * D], BF16, tag="eabf",
                                       name="eabf", bufs=6)
                    nc.gpsimd.dma_start(
                        t[:], ea_d.ap()[ci * EBLK : (ci + 1) * EBLK, :]
                    )
                else:
                    t32 = ea_pool.tile([EBLK, CHUNK_BLKS * D], F32, tag="eachunk")
                    nc.sync.dma_start(
                        t32[:], ea_d.ap()[ci * EBLK : (ci + 1) * EBLK, :]
                    )
                    t = eabf_pool.tile([EBLK, CHUNK_BLKS * D], BF16, tag="eabf")
                    nc.scalar.copy(t[:], t32[:])
                chunk_tiles[ci] = t
            return chunk_tiles[ci]

        agg_tiles = [None] * (RPC // 2 + 1)

        oh_shared = [None]

        def scatter_range(l):
            ps = sc_psum.tile([D, RW], F32, tag="scps")
            nb = int(B[l])
            for b in range(nb):
                blk = int(prefix[l]) + b
                ea_t = get_chunk(blk // CHUNK_BLKS)
                co = blk % CHUNK_BLKS
                if variant == "noheq":
                    if oh_shared[0] is None:
                        oh = oh_pool.tile([EBLK, RW], BF16, tag="oh")
                        nc.vector.tensor_scalar(
                            oh[:], iota_t[:], colrt_t[:, 0:1], None,
                            mybir.AluOpType.is_equal,
                        )
                        oh_shared[0] = oh
                    oh = oh_shared[0]
                else:
                    oh = oh_pool.tile([EBLK, RW], BF16, tag="oh")
                    nc.vector.tensor_scalar(
                        oh[:],
                        iota_t[:],
                        colrt_t[:, blk : blk + 1],
                        None,
                        mybir.AluOpType.is_equal,
                    )
                if variant == "nomm":
                    if b == 0:
                        nc.tensor.matmul(ps[:], ea_t[:, co * D : (co + 1) * D],
                                         oh[:], start=True, stop=True)
                else:
                    nc.tensor.matmul(
                        ps[:],
                        ea_t[:, co * D : (co + 1) * D],
                        oh[:],
                        start=(b == 0),
                        stop=(b == nb - 1),
                    )
            # pack two 128-node ranges into one [128, 256] agg tile so the
            # MLP agg-term matmul keeps N=256 (f32r 1 cyc/row)
            if l % 2 == 0:
                agg_tiles[l // 2] = agg_pool.tile([D, 2 * RW], F32R, tag="agg", name="aggp")
            at = agg_tiles[l // 2]
            nc.scalar.copy(at[:, (l % 2) * RW : (l % 2 + 1) * RW], ps[:])

        Relu = mybir.ActivationFunctionType.Relu

        def mlp_group(g):
            gs = g * NB_MLP
            nb = min(NB_MLP, NPC - gs)
            pairs = [j for j in (2 * g, 2 * g + 1) if j * 2 * RW < gs + nb]
            ug_t = ug_pool.tile([DG, nb], F32R, tag="ug")
            nc.scalar.dma_start(ug_t[:], ugt_d.ap()[:, gs : gs + nb])
            hs = []
            for ht in range(2):
                hp = h_psum.tile([D, nb], F32, tag="hps")
                hsl = slice(ht * D, (ht + 1) * D)
                nc.tensor.matmul(
                    hp[:], w1x_t[:, hsl], xt_t[:, gs : gs + nb], start=True, stop=False
                )
                for j in pairs:
                    o0 = j * 2 * RW - gs
                    nc.tensor.matmul(
                        hp[:, o0 : o0 + 2 * RW],
                        w1a_t[:, hsl],
                        agg_tiles[j][:],
                        start=False,
                        stop=False,
                    )
                nc.tensor.matmul(
                    hp[:], w1u_t[:, hsl], ug_t[:], start=False, stop=True
                )
                ht_sb = hs_pool.tile([D, nb], F32R, tag="hs")
                nc.scalar.activation(
                    ht_sb[:], hp[:], Relu, bias=b1t_t[:, ht : ht + 1]
                )
                hs.append(ht_sb)
            # layer 2 in transposed orientation: o2T[d, n], N=nb (f32r 1 cyc/row)
            o2 = o2_psum.tile([D, nb], F32, tag="o2ps")
            nc.tensor.matmul(o2[:], w2a_t[:], hs[0][:], start=True, stop=False)
            nc.tensor.matmul(o2[:], w2b_t[:], hs[1][:], start=False, stop=False)
            # residual: += I.T @ xT = xT
            nc.tensor.matmul(o2[:], ident_t[:], xt_t[:, gs : gs + nb],
                             start=False, stop=False)
            # bias: += b2[d] * ones[n]  (rank-1)
            nc.tensor.matmul(o2[:], b2r_t[:], ones_t[:, :nb], start=False, stop=True)
            o2_sb = o2sb_pool.tile([D, nb], F32R, tag="o2sb")
            nc.scalar.copy(o2_sb[:], o2[:])
            # transpose back to natural [n, d] via PE, 128 nodes at a time
            o_sb = os_pool.tile([D, nb], F32, tag="os")
            for nt in range(nb // D):
                nsl = slice(nt * D, (nt + 1) * D)
                tp = t_psum.tile([D, D], F32R, tag="tps")
                nc.tensor.transpose(tp[:], o2_sb[:, nsl], ident_t[:])
                nc.scalar.copy(o_sb[:, nsl], tp[:])
            dst = out_d.ap()[gs : gs + nb, :].rearrange("(b p) f -> p b f", p=EBLK)
            nc.scalar.dma_start(dst, o_sb[:].rearrange("p (b f) -> p b f", f=D))

        ngrp = (NPC + NB_MLP - 1) // NB_MLP
        loop_cm = tc.For_i(0, loop_n, 1) if loop_n > 1 else contextlib.nullcontext()
        with loop_cm:
            if variant == "dmaonly":
                dummy = persist.tile([EBLK, D], F32, tag="dummy")
                for ci in range(nchunk):
                    t = get_chunk(ci)
                    nc.vector.tensor_copy(dummy[:], t[:, 0:D].bitcast(F32))
                chunk_tiles.clear()
            else:
                for g in range(ngrp):
                    for l in (4 * g, 4 * g + 1, 4 * g + 2, 4 * g + 3):
                        if l < RPC:
                            scatter_range(l)
                    mlp_group(g)

    nc.compile()
    return nc


def kernel(**inputs) -> np.ndarray:
    in_maps, B, nchunk, nblk_alloc = _shard_inputs(
        inputs["x"], inputs["edge_index"], inputs["edge_attr"], inputs["u"],
        inputs["batch"], inputs["W1"], inputs["b1"], inputs["W2"], inputs["b2"],
    )
    nc = _build_program(B, nchunk, nblk_alloc)

    from concourse.bass_utils import run_bass_kernel_spmd

    res = run_bass_kernel_spmd(nc, in_maps, list(range(NCORES)))
    _PROFILE_RESULTS[0] = res
    out = np.concatenate([res.results[c]["out"] for c in range(NCORES)], axis=0)
    return np.ascontiguousarray(out[:N_NODES])



# revision 6
# speedup vs baseline: 1.7272x; 1.7272x over previous
"""Trainium2 Bass kernel for nn_NodeModel (GNN message passing).

Reference computation:
    agg = segment_sum(edge_attr, edge_index[1], num_segments=N)     # scatter-add
    h   = relu(concat([x, agg, u[batch]], 1) @ W1 + b1)
    out = h @ W2 + b2 + x

Strategy (8 NeuronCores, graph-parallel by destination node):
  - Nodes are sorted by in-degree (descending) and dealt round-robin across
    the 8 cores, so every core sees the same degree profile. Per core the
    12544 owned nodes form the columns of all on-chip tensors.
  - The scatter-add runs as slab adds: the host lays the k-th incoming
    edge row of every node out as "pass k" (a [128, n_k] bf16 slab,
    n_k = #nodes with degree > k; nodes sorted by degree make every pass
    a dense prefix). Pass 0 is DMAed straight into agg; passes 1.. are
    DMAed to a staging tile (HWDGE, full rate) and added into agg by DVE
    tensor_add ops (~0.5 cyc/element bf16). No per-edge compute anywhere.
  - agg is chunked into 8 column ranges with independent add chains so
    chunks pipeline; the MLP consumes chunks as their chains complete.
  - MLP: h1T[h,n] accumulated in PSUM from W1x.T xT + W1a.T aggT + W1u.T ugT;
    ReLU+bias on ScalarE during evacuation; layer 2 in transposed
    orientation with the residual (+x) as an identity matmul and the bias
    as a rank-1 matmul. Output stays transposed [d, n] bf16 on device; the
    host de-transposes and un-permutes.
  - Everything is bf16 on the wire (rel err ~5e-3 incl. bf16 accumulation).
"""

from contextlib import ExitStack

import ml_dtypes
import numpy as np

N_NODES = 100000
N_EDGES = 1600000
D = 128          # node / edge feature dim
DG = 16          # global feature dim
H = 256          # hidden dim
G = 64           # graphs
NCORES = 8

NPC = 12544      # nodes per core
N_PAD = NCORES * NPC
CW = 1568        # agg chunk width (8 chunks per core)
NCHUNK = NPC // CW
NB = 392         # MLP group columns (CW % NB == 0)
MIN_PW = 64      # minimum pass width
PW_ALIGN = 1     # pass width alignment

BF16 = ml_dtypes.bfloat16

_PROFILE_RESULTS = [None]  # stash for test harness introspection


def _plan_passes(deg, order_nodes):
    """Shared-across-cores pass widths and per-chunk DRAM layout."""
    degmat = deg[order_nodes].reshape(NPC, NCORES)      # [pos, core]
    kmax = int(degmat.max())
    ks = np.arange(1, kmax)
    # count per core of nodes with degree > k  -> max over cores
    counts = (degmat[:, :, None] > ks[None, None, :]).sum(axis=0)  # [core, k-1]
    wk = counts.max(axis=0)

    widths = [NPC]
    for k in range(1, kmax):
        w = max(int(wk[k - 1]), MIN_PW)
        w = min(-(-w // PW_ALIGN) * PW_ALIGN, NPC)
        widths.append(w)

    seg_off = np.full((max(kmax, 1), NCHUNK), -1, dtype=np.int64)
    chunk_meta = []
    base = 0
    for c in range(NCHUNK):
        lo, hi = c * CW, (c + 1) * CW
        fulls = [k for k in range(1, kmax) if widths[k] >= hi]
        partials = []
        seg_off[0, c] = base
        cur = base + CW
        for k in fulls:
            seg_off[k, c] = cur
            cur += CW
        for k in range(1, kmax):
            if lo < widths[k] < hi:
                wp = widths[k] - lo
                partials.append((k, wp, cur))
                seg_off[k, c] = cur
                cur += wp
        chunk_meta.append({"base": base, "fulls": len(fulls),
                           "partials": [(wp, off) for _, wp, off in partials]})
        base = cur
    return widths, chunk_meta, seg_off, base, kmax


def _shard_inputs(x, edge_index, edge_attr, u, batch, W1, b1, W2, b2):
    x = np.asarray(x, dtype=np.float32)
    edge_attr = np.asarray(edge_attr, dtype=np.float32)
    u = np.asarray(u, dtype=np.float32)
    batch = np.asarray(batch).astype(np.int64)
    W1 = np.asarray(W1, dtype=np.float32)
    b1 = np.asarray(b1, dtype=np.float32)
    W2 = np.asarray(W2, dtype=np.float32)
    b2 = np.asarray(b2, dtype=np.float32)
    col = np.asarray(edge_index[1]).astype(np.int64)

    deg = np.bincount(col, minlength=N_PAD).astype(np.int64)
    order_nodes = np.argsort(-deg, kind="stable")        # rank -> node id
    rank_of_node = np.empty(N_PAD, dtype=np.int64)
    rank_of_node[order_nodes] = np.arange(N_PAD)

    widths, chunk_meta, seg_off, ct, kmax = _plan_passes(deg, order_nodes)

    # --- edge slab assembly -------------------------------------------------
    r = rank_of_node[col]
    order_e = np.argsort(r, kind="stable")
    rs = r[order_e]
    cnt = np.bincount(rs, minlength=N_PAD)
    starts = np.concatenate([[0], np.cumsum(cnt)])[:-1]
    j = np.arange(N_EDGES, dtype=np.int64) - starts[rs]  # edge index within node
    core_e = rs % NCORES
    pos_e = rs // NCORES
    c_e = pos_e // CW
    dramcol = seg_off[j, c_e] + (pos_e - c_e * CW)
    assert dramcol.min() >= 0

    eap = np.zeros((NCORES, ct, D), dtype=BF16)
    eap[core_e, dramcol] = edge_attr[order_e].astype(BF16)
    eap_all = np.ascontiguousarray(eap.transpose(0, 2, 1))  # [core, 128, ct]

    # --- node-feature relayout ---------------------------------------------
    nodes_by_core = order_nodes.reshape(NPC, NCORES).T      # [core, pos]
    x_pad = np.zeros((N_PAD, D), dtype=np.float32)
    x_pad[:N_NODES] = x
    xt_all = np.ascontiguousarray(
        x_pad[nodes_by_core].transpose(0, 2, 1)).astype(BF16)
    batch_pad = np.concatenate(
        [batch, np.zeros(N_PAD - N_NODES, dtype=np.int64)])
    ug = u[batch_pad]                                       # [N_PAD, DG]
    ugt_all = np.ascontiguousarray(
        ug[nodes_by_core].transpose(0, 2, 1)).astype(BF16)

    consts = {
        "w1x": np.ascontiguousarray(W1[:D]).astype(BF16),          # [128, 256]
        "w1a": np.ascontiguousarray(W1[D:2 * D]).astype(BF16),     # [128, 256]
        "w1u": np.ascontiguousarray(W1[2 * D:]).astype(BF16),      # [16, 256]
        "b1t": np.ascontiguousarray(b1.reshape(2, D).T),           # [128, 2] f32
        "w2a": np.ascontiguousarray(W2[:D]).astype(BF16),          # [128, 128]
        "w2b": np.ascontiguousarray(W2[D:]).astype(BF16),          # [128, 128]
        "b2r": np.ascontiguousarray(b2[None, :]).astype(BF16),     # [1, 128]
        "ones": np.ones((1, NB), dtype=BF16),
        "ident": np.eye(D, dtype=np.float32).astype(BF16),
    }

    in_maps = []
    for c in range(NCORES):
        m = {"eap": eap_all[c], "xt": xt_all[c], "ugt": ugt_all[c]}
        m.update(consts)
        in_maps.append(m)
    return in_maps, chunk_meta, ct, nodes_by_core


def _build_program(chunk_meta, ct):
    import concourse.bacc as bacc
    import concourse.mybir as mybir
    import concourse.tile as tile

    F32 = mybir.dt.float32
    BF = mybir.dt.bfloat16
    Add = mybir.AluOpType.add
    Relu = mybir.ActivationFunctionType.Relu

    nc = bacc.Bacc("TRN2", target_bir_lowering=False, debug=False)

    eap_d = nc.dram_tensor("eap", [D, ct], BF, kind="ExternalInput")
    xt_d = nc.dram_tensor("xt", [D, NPC], BF, kind="ExternalInput")
    ugt_d = nc.dram_tensor("ugt", [DG, NPC], BF, kind="ExternalInput")
    w1x_d = nc.dram_tensor("w1x", [D, H], BF, kind="ExternalInput")
    w1a_d = nc.dram_tensor("w1a", [D, H], BF, kind="ExternalInput")
    w1u_d = nc.dram_tensor("w1u", [DG, H], BF, kind="ExternalInput")
    b1t_d = nc.dram_tensor("b1t", [D, 2], F32, kind="ExternalInput")
    w2a_d = nc.dram_tensor("w2a", [D, D], BF, kind="ExternalInput")
    w2b_d = nc.dram_tensor("w2b", [D, D], BF, kind="ExternalInput")
    b2r_d = nc.dram_tensor("b2r", [1, D], BF, kind="ExternalInput")
    ones_d = nc.dram_tensor("ones", [1, NB], BF, kind="ExternalInput")
    ident_d = nc.dram_tensor("ident", [D, D], BF, kind="ExternalInput")
    out_d = nc.dram_tensor("out", [D, NPC], BF, kind="ExternalOutput")

    with tile.TileContext(nc) as tc, ExitStack() as ctx:
        persist = ctx.enter_context(tc.tile_pool(name="persist", bufs=1))
        agg_pool = ctx.enter_context(tc.tile_pool(name="agg", bufs=1))
        outc_pool = ctx.enter_context(tc.tile_pool(name="outc", bufs=2))
        hs_pool = ctx.enter_context(tc.tile_pool(name="hs", bufs=4))
        h_psum = ctx.enter_context(tc.tile_pool(name="hps", bufs=4, space="PSUM"))
        o2_psum = ctx.enter_context(tc.tile_pool(name="o2ps", bufs=2, space="PSUM"))

        def pload(dram, shape, dtype):
            t = persist.tile(shape, dtype, tag=dram.name)
            nc.scalar.dma_start(t[:], dram.ap())
            return t

        w1x_t = pload(w1x_d, [D, H], BF)
        w1a_t = pload(w1a_d, [D, H], BF)
        w1u_t = pload(w1u_d, [DG, H], BF)
        b1t_t = pload(b1t_d, [D, 2], F32)
        w2a_t = pload(w2a_d, [D, D], BF)
        w2b_t = pload(w2b_d, [D, D], BF)
        b2r_t = pload(b2r_d, [1, D], BF)
        ones_t = pload(ones_d, [1, NB], BF)
        ident_t = pload(ident_d, [D, D], BF)
        xt_t = pload(xt_d, [D, NPC], BF)
        ugt_t = pload(ugt_d, [DG, NPC], BF)

        # --- scatter: HWDGE slab loads + DVE adds, per-chunk chains --------
        slab_pool = ctx.enter_context(tc.tile_pool(name="slab", bufs=8))
        agg_tiles = {}
        for c in reversed(range(NCHUNK)):
            m = chunk_meta[c]
            agg = agg_pool.tile([D, CW], BF, tag=f"agg{c}")
            agg_tiles[c] = agg
            nc.sync.dma_start(agg[:], eap_d.ap()[:, m["base"]:m["base"] + CW])
            fb = m["base"] + CW
            for i in range(m["fulls"]):
                t = slab_pool.tile([D, CW], BF, tag="slab")
                nc.sync.dma_start(t[:], eap_d.ap()[:, fb + i * CW:fb + (i + 1) * CW])
                nc.vector.tensor_add(agg[:], agg[:], t[:])
            for wp, off in m["partials"]:
                t = slab_pool.tile([D, CW], BF, tag="slab")
                nc.sync.dma_start(t[:, 0:wp], eap_d.ap()[:, off:off + wp])
                nc.vector.tensor_add(agg[:, 0:wp], agg[:, 0:wp], t[:, 0:wp])

        # --- MLP over 392-node groups, chunk by chunk ----------------------
        for c in reversed(range(NCHUNK)):
            agg = agg_tiles[c]
            outc = outc_pool.tile([D, CW], BF, tag="outc")
            for q in range(CW // NB):
                off = q * NB
                gs = c * CW + off
                hs = []
                for ht in range(2):
                    hp = h_psum.tile([D, NB], F32, tag="hp")
                    hsl = slice(ht * D, (ht + 1) * D)
                    nc.tensor.matmul(hp[:], w1x_t[:, hsl], xt_t[:, gs:gs + NB],
                                     start=True, stop=False)
                    nc.tensor.matmul(hp[:], w1a_t[:, hsl], agg[:, off:off + NB],
                                     start=False, stop=False)
                    nc.tensor.matmul(hp[:], w1u_t[:, hsl], ugt_t[:, gs:gs + NB],
                                     start=False, stop=True)
                    hsb = hs_pool.tile([D, NB], BF, tag="hs")
                    nc.scalar.activation(hsb[:], hp[:], Relu,
                                         bias=b1t_t[:, ht:ht + 1])
                    hs.append(hsb)
                o2 = o2_psum.tile([D, NB], F32, tag="o2")
                nc.tensor.matmul(o2[:], w2a_t[:], hs[0][:], start=True, stop=False)
                nc.tensor.matmul(o2[:], w2b_t[:], hs[1][:], start=False, stop=False)
                nc.tensor.matmul(o2[:], ident_t[:], xt_t[:, gs:gs + NB],
                                 start=False, stop=False)
                nc.tensor.matmul(o2[:], b2r_t[:], ones_t[:], start=False, stop=True)
                nc.scalar.copy(outc[:, off:off + NB], o2[:])
            nc.sync.dma_start(out_d.ap()[:, c * CW:(c + 1) * CW], outc[:])

    nc.compile()
    return nc


def kernel(**inputs) -> np.ndarray:
    in_maps, chunk_meta, ct, nodes_by_core = _shard_inputs(
        inputs["x"], inputs["edge_index"], inputs["edge_attr"], inputs["u"],
        inputs["batch"], inputs["W1"], inputs["b1"], inputs["W2"], inputs["b2"],
    )
    nc = _build_program(chunk_meta, ct)

    from concourse.bass_utils import run_bass_kernel_spmd

    res = run_bass_kernel_spmd(nc, in_maps, list(range(NCORES)))
    _PROFILE_RESULTS[0] = res
    full = np.empty((N_PAD, D), dtype=np.float32)
    for c in range(NCORES):
        full[nodes_by_core[c]] = res.results[c]["out"].astype(np.float32).T
    return np.ascontiguousarray(full[:N_NODES])


# revision 8
# speedup vs baseline: 1.9305x; 1.1178x over previous
"""Trainium2 Bass kernel for nn_NodeModel (GNN message passing).

Reference computation:
    agg = segment_sum(edge_attr, edge_index[1], num_segments=N)     # scatter-add
    h   = relu(concat([x, agg, u[batch]], 1) @ W1 + b1)
    out = h @ W2 + b2 + x

Strategy (8 NeuronCores, graph-parallel by destination node):
  - Nodes are sorted by in-degree (descending) and dealt round-robin across
    the 8 cores, so every core sees the same degree profile. Per core the
    12544 owned nodes form the columns of all on-chip tensors.
  - The scatter-add runs as slab adds: the host lays the k-th incoming
    edge row of every node out as "pass k" (a [128, n_k] bf16 slab,
    n_k = #nodes with degree > k; nodes sorted by degree make every pass
    a dense prefix). Pass 0 is DMAed straight into agg; passes 1.. are
    DMAed to a staging tile (HWDGE, full rate) and added into agg by DVE
    tensor_add ops (~0.5 cyc/element bf16). No per-edge compute anywhere.
  - agg is chunked into 8 column ranges with independent add chains so
    chunks pipeline; the MLP consumes chunks as their chains complete.
  - MLP: h1T[h,n] accumulated in PSUM from W1x.T xT + W1a.T aggT + W1u.T ugT;
    ReLU+bias on ScalarE during evacuation; layer 2 in transposed
    orientation with the residual (+x) as an identity matmul and the bias
    as a rank-1 matmul. Output stays transposed [d, n] bf16 on device; the
    host de-transposes and un-permutes.
  - Everything is bf16 on the wire (rel err ~5e-3 incl. bf16 accumulation).
"""

from contextlib import ExitStack

import ml_dtypes
import numpy as np

N_NODES = 100000
N_EDGES = 1600000
D = 128          # node / edge feature dim
DG = 16          # global feature dim
H = 256          # hidden dim
G = 64           # graphs
NCORES = 8

NPC = 12544      # nodes per core
N_PAD = NCORES * NPC
CW = 1568        # agg chunk width (8 chunks per core)
NCHUNK = NPC // CW
NB = 392         # MLP group columns (CW % NB == 0)
MIN_PW = 64      # minimum pass width
PW_ALIGN = 1     # pass width alignment

BF16 = ml_dtypes.bfloat16

_PROFILE_RESULTS = [None]  # stash for test harness introspection


def _plan_passes(deg, order_nodes):
    """Shared-across-cores pass widths and per-chunk DRAM layout."""
    degmat = deg[order_nodes].reshape(NPC, NCORES)      # [pos, core]
    kmax = int(degmat.max())
    ks = np.arange(1, kmax)
    # count per core of nodes with degree > k  -> max over cores
    counts = (degmat[:, :, None] > ks[None, None, :]).sum(axis=0)  # [core, k-1]
    wk = counts.max(axis=0)

    widths = [NPC]
    for k in range(1, kmax):
        w = max(int(wk[k - 1]), MIN_PW)
        w = min(-(-w // PW_ALIGN) * PW_ALIGN, NPC)
        widths.append(w)

    seg_off = np.full((max(kmax, 1), NCHUNK), -1, dtype=np.int64)
    chunk_meta = []
    base = 0
    for c in range(NCHUNK):
        lo, hi = c * CW, (c + 1) * CW
        fulls = [k for k in range(1, kmax) if widths[k] >= hi]
        partials = []
        seg_off[0, c] = base
        cur = base + CW
        for k in fulls:
            seg_off[k, c] = cur
            cur += CW
        for k in range(1, kmax):
            if lo < widths[k] < hi:
                wp = widths[k] - lo
                partials.append((k, wp, cur))
                seg_off[k, c] = cur
                cur += wp
        chunk_meta.append({"base": base, "fulls": len(fulls),
                           "partials": [(wp, off) for _, wp, off in partials]})
        base = cur
    return widths, chunk_meta, seg_off, base, kmax


def _shard_inputs(x, edge_index, edge_attr, u, batch, W1, b1, W2, b2):
    x = np.asarray(x, dtype=np.float32)
    edge_attr = np.asarray(edge_attr, dtype=np.float32)
    u = np.asarray(u, dtype=np.float32)
    batch = np.asarray(batch).astype(np.int64)
    W1 = np.asarray(W1, dtype=np.float32)
    b1 = np.asarray(b1, dtype=np.float32)
    W2 = np.asarray(W2, dtype=np.float32)
    b2 = np.asarray(b2, dtype=np.float32)
    col = np.asarray(edge_index[1]).astype(np.int64)

    deg = np.bincount(col, minlength=N_PAD).astype(np.int64)
    order_nodes = np.argsort(-deg, kind="stable")        # rank -> node id
    rank_of_node = np.empty(N_PAD, dtype=np.int64)
    rank_of_node[order_nodes] = np.arange(N_PAD)

    widths, chunk_meta, seg_off, ct, kmax = _plan_passes(deg, order_nodes)

    # --- edge slab assembly -------------------------------------------------
    r = rank_of_node[col]
    order_e = np.argsort(r, kind="stable")
    rs = r[order_e]
    cnt = np.bincount(rs, minlength=N_PAD)
    starts = np.concatenate([[0], np.cumsum(cnt)])[:-1]
    j = np.arange(N_EDGES, dtype=np.int64) - starts[rs]  # edge index within node
    core_e = rs % NCORES
    pos_e = rs // NCORES
    c_e = pos_e // CW
    dramcol = seg_off[j, c_e] + (pos_e - c_e * CW)
    assert dramcol.min() >= 0

    eap = np.zeros((NCORES, ct, D), dtype=BF16)
    eap[core_e, dramcol] = edge_attr[order_e].astype(BF16)
    eap_all = np.ascontiguousarray(eap.transpose(0, 2, 1))  # [core, 128, ct]

    # --- node-feature relayout ---------------------------------------------
    nodes_by_core = order_nodes.reshape(NPC, NCORES).T      # [core, pos]
    x_pad = np.zeros((N_PAD, D), dtype=np.float32)
    x_pad[:N_NODES] = x
    xt_all = np.ascontiguousarray(
        x_pad[nodes_by_core].transpose(0, 2, 1)).astype(BF16)
    batch_pad = np.concatenate(
        [batch, np.zeros(N_PAD - N_NODES, dtype=np.int64)])
    ug = u[batch_pad]                                       # [N_PAD, DG]
    ugt_all = np.ascontiguousarray(
        ug[nodes_by_core].transpose(0, 2, 1)).astype(BF16)

    consts = {
        "w1x": np.ascontiguousarray(W1[:D]).astype(BF16),          # [128, 256]
        "w1a": np.ascontiguousarray(W1[D:2 * D]).astype(BF16),     # [128, 256]
        "w1u": np.ascontiguousarray(W1[2 * D:]).astype(BF16),      # [16, 256]
        "b1t": np.ascontiguousarray(b1.reshape(2, D).T),           # [128, 2] f32
        "w2a": np.ascontiguousarray(W2[:D]).astype(BF16),          # [128, 128]
        "w2b": np.ascontiguousarray(W2[D:]).astype(BF16),          # [128, 128]
        "b2r": np.ascontiguousarray(b2[None, :]).astype(BF16),     # [1, 128]
        "ones": np.ones((1, NB), dtype=BF16),
        "ident": np.eye(D, dtype=np.float32).astype(BF16),
    }

    in_maps = []
    for c in range(NCORES):
        m = {"eap": eap_all[c], "xt": xt_all[c], "ugt": ugt_all[c]}
        m.update(consts)
        in_maps.append(m)
    return in_maps, chunk_meta, ct, nodes_by_core


def _build_program(chunk_meta, ct):
    import concourse.bacc as bacc
    import concourse.mybir as mybir
    import concourse.tile as tile

    F32 = mybir.dt.float32
    BF = mybir.dt.bfloat16
    Add = mybir.AluOpType.add
    Relu = mybir.ActivationFunctionType.Relu

    nc = bacc.Bacc("TRN2", target_bir_lowering=False, debug=False)

    eap_d = nc.dram_tensor("eap", [D, ct], BF, kind="ExternalInput")
    xt_d = nc.dram_tensor("xt", [D, NPC], BF, kind="ExternalInput")
    ugt_d = nc.dram_tensor("ugt", [DG, NPC], BF, kind="ExternalInput")
    w1x_d = nc.dram_tensor("w1x", [D, H], BF, kind="ExternalInput")
    w1a_d = nc.dram_tensor("w1a", [D, H], BF, kind="ExternalInput")
    w1u_d = nc.dram_tensor("w1u", [DG, H], BF, kind="ExternalInput")
    b1t_d = nc.dram_tensor("b1t", [D, 2], F32, kind="ExternalInput")
    w2a_d = nc.dram_tensor("w2a", [D, D], BF, kind="ExternalInput")
    w2b_d = nc.dram_tensor("w2b", [D, D], BF, kind="ExternalInput")
    b2r_d = nc.dram_tensor("b2r", [1, D], BF, kind="ExternalInput")
    ones_d = nc.dram_tensor("ones", [1, NB], BF, kind="ExternalInput")
    ident_d = nc.dram_tensor("ident", [D, D], BF, kind="ExternalInput")
    out_d = nc.dram_tensor("out", [D, NPC], BF, kind="ExternalOutput")

    with tile.TileContext(nc) as tc, ExitStack() as ctx:
        persist = ctx.enter_context(tc.tile_pool(name="persist", bufs=1))
        agg_pool = ctx.enter_context(tc.tile_pool(name="agg", bufs=1))
        outc_pool = ctx.enter_context(tc.tile_pool(name="outc", bufs=NCHUNK))
        hs_pool = ctx.enter_context(tc.tile_pool(name="hs", bufs=4))
        h_psum = ctx.enter_context(tc.tile_pool(name="hps", bufs=4, space="PSUM"))
        o2_psum = ctx.enter_context(tc.tile_pool(name="o2ps", bufs=2, space="PSUM"))

        def pload(dram, shape, dtype):
            t = persist.tile(shape, dtype, tag=dram.name)
            nc.scalar.dma_start(t[:], dram.ap())
            return t

        w1x_t = pload(w1x_d, [D, H], BF)
        w1a_t = pload(w1a_d, [D, H], BF)
        w1u_t = pload(w1u_d, [DG, H], BF)
        b1t_t = pload(b1t_d, [D, 2], F32)
        w2a_t = pload(w2a_d, [D, D], BF)
        w2b_t = pload(w2b_d, [D, D], BF)
        b2r_t = pload(b2r_d, [1, D], BF)
        ones_t = pload(ones_d, [1, NB], BF)
        ident_t = pload(ident_d, [D, D], BF)
        xt_t = pload(xt_d, [D, NPC], BF)
        ugt_t = pload(ugt_d, [DG, NPC], BF)

        # --- scatter: HWDGE slab loads + DVE adds, per-chunk chains --------
        slab_pool = ctx.enter_context(tc.tile_pool(name="slab", bufs=8))
        agg_tiles = {}
        for c in reversed(range(NCHUNK)):
            m = chunk_meta[c]
            agg = agg_pool.tile([D, CW], BF, tag=f"agg{c}")
            agg_tiles[c] = agg
            nc.sync.dma_start(agg[:], eap_d.ap()[:, m["base"]:m["base"] + CW])
            fb = m["base"] + CW
            for i in range(m["fulls"]):
                t = slab_pool.tile([D, CW], BF, tag="slab")
                nc.sync.dma_start(t[:], eap_d.ap()[:, fb + i * CW:fb + (i + 1) * CW])
                nc.vector.tensor_add(agg[:], agg[:], t[:])
            for wp, off in m["partials"]:
                t = slab_pool.tile([D, CW], BF, tag="slab")
                nc.sync.dma_start(t[:, 0:wp], eap_d.ap()[:, off:off + wp])
                nc.vector.tensor_add(agg[:, 0:wp], agg[:, 0:wp], t[:, 0:wp])

        # --- MLP over 392-node groups, chunk by chunk ----------------------
        for c in reversed(range(NCHUNK)):
            agg = agg_tiles[c]
            outc = outc_pool.tile([D, CW], BF, tag="outc")
            for q in range(CW // NB):
                off = q * NB
                gs = c * CW + off
                hs = []
                for ht in range(2):
                    hp = h_psum.tile([D, NB], F32, tag="hp")
                    hsl = slice(ht * D, (ht + 1) * D)
                    nc.tensor.matmul(hp[:], w1x_t[:, hsl], xt_t[:, gs:gs + NB],
                                     start=True, stop=False)
                    nc.tensor.matmul(hp[:], w1a_t[:, hsl], agg[:, off:off + NB],
                                     start=False, stop=False)
                    nc.tensor.matmul(hp[:], w1u_t[:, hsl], ugt_t[:, gs:gs + NB],
                                     start=False, stop=True)
                    hsb = hs_pool.tile([D, NB], BF, tag="hs")
                    nc.scalar.activation(hsb[:], hp[:], Relu,
                                         bias=b1t_t[:, ht:ht + 1])
                    hs.append(hsb)
                o2 = o2_psum.tile([D, NB], F32, tag="o2")
                nc.tensor.matmul(o2[:], w2a_t[:], hs[0][:], start=True, stop=False)
                nc.tensor.matmul(o2[:], w2b_t[:], hs[1][:], start=False, stop=False)
                nc.tensor.matmul(o2[:], ident_t[:], xt_t[:, gs:gs + NB],
                                 start=False, stop=False)
                nc.tensor.matmul(o2[:], b2r_t[:], ones_t[:], start=False, stop=True)
                nc.scalar.copy(outc[:, off:off + NB], o2[:])
            nc.scalar.dma_start(out_d.ap()[:, c * CW:(c + 1) * CW], outc[:])

    nc.compile()
    return nc


def kernel(**inputs) -> np.ndarray:
    in_maps, chunk_meta, ct, nodes_by_core = _shard_inputs(
        inputs["x"], inputs["edge_index"], inputs["edge_attr"], inputs["u"],
        inputs["batch"], inputs["W1"], inputs["b1"], inputs["W2"], inputs["b2"],
    )
    nc = _build_program(chunk_meta, ct)

    from concourse.bass_utils import run_bass_kernel_spmd

    res = run_bass_kernel_spmd(nc, in_maps, list(range(NCORES)))
    _PROFILE_RESULTS[0] = res
    full = np.empty((N_PAD, D), dtype=np.float32)
    for c in range(NCORES):
        full[nodes_by_core[c]] = res.results[c]["out"].astype(np.float32).T
    return np.ascontiguousarray(full[:N_NODES])


# revision 9
# speedup vs baseline: 2.1252x; 1.1008x over previous
"""Trainium2 Bass kernel for nn_NodeModel (GNN message passing).

Reference computation:
    agg = segment_sum(edge_attr, edge_index[1], num_segments=N)     # scatter-add
    h   = relu(concat([x, agg, u[batch]], 1) @ W1 + b1)
    out = h @ W2 + b2 + x

Strategy (8 NeuronCores, graph-parallel by destination node):
  - Nodes are sorted by in-degree (descending) and dealt round-robin across
    the 8 cores, so every core sees the same degree profile. Per core the
    12544 owned nodes form the columns of all on-chip tensors.
  - The scatter-add runs as slab adds: the host lays the k-th incoming
    edge row of every node out as "pass k" (a [128, n_k] bf16 slab,
    n_k = #nodes with degree > k; nodes sorted by degree make every pass
    a dense prefix). Pass 0 is DMAed straight into agg; passes 1.. are
    DMAed to a staging tile (HWDGE, full rate) and added into agg by DVE
    tensor_add ops (~0.5 cyc/element bf16). No per-edge compute anywhere.
  - agg is chunked into 8 column ranges with independent add chains so
    chunks pipeline; the MLP consumes chunks as their chains complete.
  - MLP: h1T[h,n] accumulated in PSUM from W1x.T xT + W1a.T aggT + W1u.T ugT;
    ReLU+bias on ScalarE during evacuation; layer 2 in transposed
    orientation with the residual (+x) as an identity matmul and the bias
    as a rank-1 matmul. Output stays transposed [d, n] bf16 on device; the
    host de-transposes and un-permutes.
  - Everything is bf16 on the wire (rel err ~5e-3 incl. bf16 accumulation).
"""

from contextlib import ExitStack

import ml_dtypes
import numpy as np

N_NODES = 100000
N_EDGES = 1600000
D = 128          # node / edge feature dim
DG = 16          # global feature dim
H = 256          # hidden dim
G = 64           # graphs
NCORES = 8

NPC = 12544      # nodes per core
N_PAD = NCORES * NPC
CW = 1568        # agg chunk width (8 chunks per core)
NCHUNK = NPC // CW
NB = 392         # MLP group columns (CW % NB == 0)
MIN_PW = 64      # minimum pass width
PW_ALIGN = 1     # pass width alignment

BF16 = ml_dtypes.bfloat16

_PROFILE_RESULTS = [None]  # stash for test harness introspection


def _plan_passes(deg, order_nodes):
    """Shared-across-cores pass widths and per-chunk DRAM layout."""
    degmat = deg[order_nodes].reshape(NPC, NCORES)      # [pos, core]
    kmax = int(degmat.max())
    ks = np.arange(1, kmax)
    # count per core of nodes with degree > k  -> max over cores
    counts = (degmat[:, :, None] > ks[None, None, :]).sum(axis=0)  # [core, k-1]
    wk = counts.max(axis=0)

    widths = [NPC]
    for k in range(1, kmax):
        w = max(int(wk[k - 1]), MIN_PW)
        w = min(-(-w // PW_ALIGN) * PW_ALIGN, NPC)
        widths.append(w)

    seg_off = np.full((max(kmax, 1), NCHUNK), -1, dtype=np.int64)
    chunk_meta = []
    base = 0
    for c in range(NCHUNK):
        lo, hi = c * CW, (c + 1) * CW
        fulls = [k for k in range(1, kmax) if widths[k] >= hi]
        partials = []
        seg_off[0, c] = base
        cur = base + CW
        for k in fulls:
            seg_off[k, c] = cur
            cur += CW
        for k in range(1, kmax):
            if lo < widths[k] < hi:
                wp = widths[k] - lo
                partials.append((k, wp, cur))
                seg_off[k, c] = cur
                cur += wp
        chunk_meta.append({"base": base, "fulls": len(fulls),
                           "partials": [(wp, off) for _, wp, off in partials]})
        base = cur
    return widths, chunk_meta, seg_off, base, kmax


def _shard_inputs(x, edge_index, edge_attr, u, batch, W1, b1, W2, b2):
    x = np.asarray(x, dtype=np.float32)
    edge_attr = np.asarray(edge_attr, dtype=np.float32)
    u = np.asarray(u, dtype=np.float32)
    batch = np.asarray(batch).astype(np.int64)
    W1 = np.asarray(W1, dtype=np.float32)
    b1 = np.asarray(b1, dtype=np.float32)
    W2 = np.asarray(W2, dtype=np.float32)
    b2 = np.asarray(b2, dtype=np.float32)
    col = np.asarray(edge_index[1]).astype(np.int64)

    deg = np.bincount(col, minlength=N_PAD).astype(np.int64)
    order_nodes = np.argsort(-deg, kind="stable")        # rank -> node id
    rank_of_node = np.empty(N_PAD, dtype=np.int64)
    rank_of_node[order_nodes] = np.arange(N_PAD)

    widths, chunk_meta, seg_off, ct, kmax = _plan_passes(deg, order_nodes)

    # --- edge slab assembly -------------------------------------------------
    r = rank_of_node[col]
    order_e = np.argsort(r, kind="stable")
    rs = r[order_e]
    cnt = np.bincount(rs, minlength=N_PAD)
    starts = np.concatenate([[0], np.cumsum(cnt)])[:-1]
    j = np.arange(N_EDGES, dtype=np.int64) - starts[rs]  # edge index within node
    core_e = rs % NCORES
    pos_e = rs // NCORES
    c_e = pos_e // CW
    dramcol = seg_off[j, c_e] + (pos_e - c_e * CW)
    assert dramcol.min() >= 0

    eap = np.zeros((NCORES, ct, D), dtype=BF16)
    eap[core_e, dramcol] = edge_attr[order_e].astype(BF16)
    eap_all = np.ascontiguousarray(eap.transpose(0, 2, 1))  # [core, 128, ct]

    # --- node-feature relayout ---------------------------------------------
    nodes_by_core = order_nodes.reshape(NPC, NCORES).T      # [core, pos]
    x_pad = np.zeros((N_PAD, D), dtype=np.float32)
    x_pad[:N_NODES] = x
    xt_all = np.ascontiguousarray(
        x_pad[nodes_by_core].transpose(0, 2, 1)).astype(BF16)
    batch_pad = np.concatenate(
        [batch, np.zeros(N_PAD - N_NODES, dtype=np.int64)])
    ug = u[batch_pad]                                       # [N_PAD, DG]
    ugt_all = np.ascontiguousarray(
        ug[nodes_by_core].transpose(0, 2, 1)).astype(BF16)

    consts = {
        "w1x": np.ascontiguousarray(W1[:D]).astype(BF16),          # [128, 256]
        "w1a": np.ascontiguousarray(W1[D:2 * D]).astype(BF16),     # [128, 256]
        "w1u": np.ascontiguousarray(W1[2 * D:]).astype(BF16),      # [16, 256]
        "b1t": np.ascontiguousarray(b1.reshape(2, D).T),           # [128, 2] f32
        "w2a": np.ascontiguousarray(W2[:D]).astype(BF16),          # [128, 128]
        "w2b": np.ascontiguousarray(W2[D:]).astype(BF16),          # [128, 128]
        "b2r": np.ascontiguousarray(b2[None, :]).astype(BF16),     # [1, 128]
        "ones": np.ones((1, NB), dtype=BF16),
        "ident": np.eye(D, dtype=np.float32).astype(BF16),
    }

    in_maps = []
    for c in range(NCORES):
        m = {"eap": eap_all[c], "xt": xt_all[c], "ugt": ugt_all[c]}
        m.update(consts)
        in_maps.append(m)
    return in_maps, chunk_meta, ct, nodes_by_core


def _build_program(chunk_meta, ct):
    import concourse.bacc as bacc
    import concourse.mybir as mybir
    import concourse.tile as tile

    F32 = mybir.dt.float32
    BF = mybir.dt.bfloat16
    Add = mybir.AluOpType.add
    Relu = mybir.ActivationFunctionType.Relu

    nc = bacc.Bacc("TRN2", target_bir_lowering=False, debug=False)

    eap_d = nc.dram_tensor("eap", [D, ct], BF, kind="ExternalInput")
    xt_d = nc.dram_tensor("xt", [D, NPC], BF, kind="ExternalInput")
    ugt_d = nc.dram_tensor("ugt", [DG, NPC], BF, kind="ExternalInput")
    w1x_d = nc.dram_tensor("w1x", [D, H], BF, kind="ExternalInput")
    w1a_d = nc.dram_tensor("w1a", [D, H], BF, kind="ExternalInput")
    w1u_d = nc.dram_tensor("w1u", [DG, H], BF, kind="ExternalInput")
    b1t_d = nc.dram_tensor("b1t", [D, 2], F32, kind="ExternalInput")
    w2a_d = nc.dram_tensor("w2a", [D, D], BF, kind="ExternalInput")
    w2b_d = nc.dram_tensor("w2b", [D, D], BF, kind="ExternalInput")
    b2r_d = nc.dram_tensor("b2r", [1, D], BF, kind="ExternalInput")
    ones_d = nc.dram_tensor("ones", [1, NB], BF, kind="ExternalInput")
    ident_d = nc.dram_tensor("ident", [D, D], BF, kind="ExternalInput")
    out_d = nc.dram_tensor("out", [D, NPC], BF, kind="ExternalOutput")

    with tile.TileContext(nc) as tc, ExitStack() as ctx:
        persist = ctx.enter_context(tc.tile_pool(name="persist", bufs=1))
        agg_pool = ctx.enter_context(tc.tile_pool(name="agg", bufs=1))
        outc_pool = ctx.enter_context(tc.tile_pool(name="outc", bufs=NCHUNK))
        hs_pool = ctx.enter_context(tc.tile_pool(name="hs", bufs=4))
        h_psum = ctx.enter_context(tc.tile_pool(name="hps", bufs=4, space="PSUM"))
        o2_psum = ctx.enter_context(tc.tile_pool(name="o2ps", bufs=2, space="PSUM"))

        def pload(dram, shape, dtype):
            t = persist.tile(shape, dtype, tag=dram.name)
            nc.scalar.dma_start(t[:], dram.ap())
            return t

        w1x_t = pload(w1x_d, [D, H], BF)
        w1a_t = pload(w1a_d, [D, H], BF)
        w1u_t = pload(w1u_d, [DG, H], BF)
        b1t_t = pload(b1t_d, [D, 2], F32)
        w2a_t = pload(w2a_d, [D, D], BF)
        w2b_t = pload(w2b_d, [D, D], BF)
        b2r_t = pload(b2r_d, [1, D], BF)
        ones_t = pload(ones_d, [1, NB], BF)
        ident_t = pload(ident_d, [D, D], BF)
        xt_t = pload(xt_d, [D, NPC], BF)
        ugt_t = pload(ugt_d, [DG, NPC], BF)

        # --- scatter: HWDGE slab loads + DVE adds, per-chunk chains --------
        slab_pool = ctx.enter_context(tc.tile_pool(name="slab", bufs=16))
        agg_tiles = {}
        for c in range(NCHUNK):
            m = chunk_meta[c]
            agg = agg_pool.tile([D, CW], BF, tag=f"agg{c}")
            agg_tiles[c] = agg
            nc.sync.dma_start(agg[:], eap_d.ap()[:, m["base"]:m["base"] + CW])
            fb = m["base"] + CW
            for i in range(m["fulls"]):
                t = slab_pool.tile([D, CW], BF, tag="slab")
                nc.sync.dma_start(t[:], eap_d.ap()[:, fb + i * CW:fb + (i + 1) * CW])
                nc.vector.tensor_add(agg[:], agg[:], t[:])
            for wp, off in m["partials"]:
                t = slab_pool.tile([D, CW], BF, tag="slab")
                nc.sync.dma_start(t[:, 0:wp], eap_d.ap()[:, off:off + wp])
                nc.vector.tensor_add(agg[:, 0:wp], agg[:, 0:wp], t[:, 0:wp])

        # --- MLP over 392-node groups, chunk by chunk ----------------------
        for c in range(NCHUNK):
            agg = agg_tiles[c]
            outc = outc_pool.tile([D, CW], BF, tag="outc")
            for q in range(CW // NB):
                off = q * NB
                gs = c * CW + off
                hs = []
                for ht in range(2):
                    hp = h_psum.tile([D, NB], F32, tag="hp")
                    hsl = slice(ht * D, (ht + 1) * D)
                    nc.tensor.matmul(hp[:], w1x_t[:, hsl], xt_t[:, gs:gs + NB],
                                     start=True, stop=False)
                    nc.tensor.matmul(hp[:], w1a_t[:, hsl], agg[:, off:off + NB],
                                     start=False, stop=False)
                    nc.tensor.matmul(hp[:], w1u_t[:, hsl], ugt_t[:, gs:gs + NB],
                                     start=False, stop=True)
                    hsb = hs_pool.tile([D, NB], BF, tag="hs")
                    nc.scalar.activation(hsb[:], hp[:], Relu,
                                         bias=b1t_t[:, ht:ht + 1])
                    hs.append(hsb)
                o2 = o2_psum.tile([D, NB], F32, tag="o2")
                nc.tensor.matmul(o2[:], w2a_t[:], hs[0][:], start=True, stop=False)
                nc.tensor.matmul(o2[:], w2b_t[:], hs[1][:], start=False, stop=False)
                nc.tensor.matmul(o2[:], ident_t[:], xt_t[:, gs:gs + NB],
                                 start=False, stop=False)
                nc.tensor.matmul(o2[:], b2r_t[:], ones_t[:], start=False, stop=True)
                nc.scalar.copy(outc[:, off:off + NB], o2[:])
            nc.scalar.dma_start(out_d.ap()[:, c * CW:(c + 1) * CW], outc[:])

    nc.compile()
    return nc


def kernel(**inputs) -> np.ndarray:
    in_maps, chunk_meta, ct, nodes_by_core = _shard_inputs(
        inputs["x"], inputs["edge_index"], inputs["edge_attr"], inputs["u"],
        inputs["batch"], inputs["W1"], inputs["b1"], inputs["W2"], inputs["b2"],
    )
    nc = _build_program(chunk_meta, ct)

    from concourse.bass_utils import run_bass_kernel_spmd

    res = run_bass_kernel_spmd(nc, in_maps, list(range(NCORES)))
    _PROFILE_RESULTS[0] = res
    full = np.empty((N_PAD, D), dtype=np.float32)
    for c in range(NCORES):
        full[nodes_by_core[c]] = res.results[c]["out"].astype(np.float32).T
    return np.ascontiguousarray(full[:N_NODES])


# revision 10
# speedup vs baseline: 2.2535x; 1.0603x over previous
"""Trainium2 Bass kernel for nn_NodeModel (GNN message passing).

Reference computation:
    agg = segment_sum(edge_attr, edge_index[1], num_segments=N)     # scatter-add
    h   = relu(concat([x, agg, u[batch]], 1) @ W1 + b1)
    out = h @ W2 + b2 + x

Strategy (8 NeuronCores, graph-parallel by destination node):
  - Nodes are sorted by in-degree (descending) and dealt round-robin across
    the 8 cores, so every core sees the same degree profile. Per core the
    12544 owned nodes form the columns of all on-chip tensors.
  - The scatter-add runs as slab adds: the host lays the k-th incoming
    edge row of every node out as "pass k" (a [128, n_k] bf16 slab,
    n_k = #nodes with degree > k; nodes sorted by degree make every pass
    a dense prefix). Pass 0 is DMAed straight into agg; passes 1.. are
    DMAed to a staging tile (HWDGE, full rate) and added into agg by DVE
    tensor_add ops (~0.5 cyc/element bf16). No per-edge compute anywhere.
  - agg is chunked into 8 column ranges with independent add chains so
    chunks pipeline; the MLP consumes chunks as their chains complete.
  - MLP: h1T[h,n] accumulated in PSUM from W1x.T xT + W1a.T aggT + W1u.T ugT;
    ReLU+bias on ScalarE during evacuation; layer 2 in transposed
    orientation with the residual (+x) as an identity matmul and the bias
    as a rank-1 matmul. Output stays transposed [d, n] bf16 on device; the
    host de-transposes and un-permutes.
  - Everything is bf16 on the wire (rel err ~5e-3 incl. bf16 accumulation).
"""

from contextlib import ExitStack

import ml_dtypes
import numpy as np

N_NODES = 100000
N_EDGES = 1600000
D = 128          # node / edge feature dim
DG = 16          # global feature dim
H = 256          # hidden dim
G = 64           # graphs
NCORES = 8

NPC = 12544      # nodes per core
N_PAD = NCORES * NPC
CW = 1568        # agg chunk width (8 chunks per core)
NCHUNK = NPC // CW
NB = 392         # MLP group columns (CW % NB == 0)
MIN_PW = 64      # minimum pass width
PW_ALIGN = 1     # pass width alignment

BF16 = ml_dtypes.bfloat16

_PROFILE_RESULTS = [None]  # stash for test harness introspection


def _plan_passes(deg, order_nodes):
    """Shared-across-cores pass widths and per-chunk DRAM layout."""
    degmat = deg[order_nodes].reshape(NPC, NCORES)      # [pos, core]
    kmax = int(degmat.max())
    ks = np.arange(1, kmax)
    # count per core of nodes with degree > k  -> max over cores
    counts = (degmat[:, :, None] > ks[None, None, :]).sum(axis=0)  # [core, k-1]
    wk = counts.max(axis=0)

    widths = [NPC]
    for k in range(1, kmax):
        w = max(int(wk[k - 1]), MIN_PW)
        w = min(-(-w // PW_ALIGN) * PW_ALIGN, NPC)
        widths.append(w)

    seg_off = np.full((max(kmax, 1), NCHUNK), -1, dtype=np.int64)
    chunk_meta = []
    base = 0
    for c in range(NCHUNK):
        lo, hi = c * CW, (c + 1) * CW
        fulls = [k for k in range(1, kmax) if widths[k] >= hi]
        partials = []
        seg_off[0, c] = base
        cur = base + CW
        for k in fulls:
            seg_off[k, c] = cur
            cur += CW
        for k in range(1, kmax):
            if lo < widths[k] < hi:
                wp = widths[k] - lo
                partials.append((k, wp, cur))
                seg_off[k, c] = cur
                cur += wp
        chunk_meta.append({"base": base, "fulls": len(fulls),
                           "partials": [(wp, off) for _, wp, off in partials]})
        base = cur
    return widths, chunk_meta, seg_off, base, kmax


def _shard_inputs(x, edge_index, edge_attr, u, batch, W1, b1, W2, b2):
    x = np.asarray(x, dtype=np.float32)
    edge_attr = np.asarray(edge_attr, dtype=np.float32)
    u = np.asarray(u, dtype=np.float32)
    batch = np.asarray(batch).astype(np.int64)
    W1 = np.asarray(W1, dtype=np.float32)
    b1 = np.asarray(b1, dtype=np.float32)
    W2 = np.asarray(W2, dtype=np.float32)
    b2 = np.asarray(b2, dtype=np.float32)
    col = np.asarray(edge_index[1]).astype(np.int64)

    deg = np.bincount(col, minlength=N_PAD).astype(np.int64)
    order_nodes = np.argsort(-deg, kind="stable")        # rank -> node id
    rank_of_node = np.empty(N_PAD, dtype=np.int64)
    rank_of_node[order_nodes] = np.arange(N_PAD)

    widths, chunk_meta, seg_off, ct, kmax = _plan_passes(deg, order_nodes)

    # --- edge slab assembly -------------------------------------------------
    r = rank_of_node[col]
    order_e = np.argsort(r, kind="stable")
    rs = r[order_e]
    cnt = np.bincount(rs, minlength=N_PAD)
    starts = np.concatenate([[0], np.cumsum(cnt)])[:-1]
    j = np.arange(N_EDGES, dtype=np.int64) - starts[rs]  # edge index within node
    core_e = rs % NCORES
    pos_e = rs // NCORES
    c_e = pos_e // CW
    dramcol = seg_off[j, c_e] + (pos_e - c_e * CW)
    assert dramcol.min() >= 0

    eap = np.zeros((NCORES, ct, D), dtype=BF16)
    eap[core_e, dramcol] = edge_attr[order_e].astype(BF16)
    eap_all = np.ascontiguousarray(eap.transpose(0, 2, 1))  # [core, 128, ct]

    # --- node-feature relayout ---------------------------------------------
    nodes_by_core = order_nodes.reshape(NPC, NCORES).T      # [core, pos]
    x_pad = np.zeros((N_PAD, D), dtype=np.float32)
    x_pad[:N_NODES] = x
    xt_all = np.ascontiguousarray(
        x_pad[nodes_by_core].transpose(0, 2, 1)).astype(BF16)
    batch_pad = np.concatenate(
        [batch, np.zeros(N_PAD - N_NODES, dtype=np.int64)])
    ug = u[batch_pad]                                       # [N_PAD, DG]
    ugt_all = np.ascontiguousarray(
        ug[nodes_by_core].transpose(0, 2, 1)).astype(BF16)

    consts = {
        "w1x": np.ascontiguousarray(W1[:D]).astype(BF16),          # [128, 256]
        "w1a": np.ascontiguousarray(W1[D:2 * D]).astype(BF16),     # [128, 256]
        "w1u": np.ascontiguousarray(W1[2 * D:]).astype(BF16),      # [16, 256]
        "b1t": np.ascontiguousarray(b1.reshape(2, D).T),           # [128, 2] f32
        "w2a": np.ascontiguousarray(W2[:D]).astype(BF16),          # [128, 128]
        "w2b": np.ascontiguousarray(W2[D:]).astype(BF16),          # [128, 128]
        "b2r": np.ascontiguousarray(b2[None, :]).astype(BF16),     # [1, 128]
        "ones": np.ones((1, NB), dtype=BF16),
        "ident": np.eye(D, dtype=np.float32).astype(BF16),
    }

    in_maps = []
    for c in range(NCORES):
        m = {"eap": eap_all[c], "xt": xt_all[c], "ugt": ugt_all[c]}
        m.update(consts)
        in_maps.append(m)
    return in_maps, chunk_meta, ct, nodes_by_core


def _build_program(chunk_meta, ct):
    import concourse.bacc as bacc
    import concourse.mybir as mybir
    import concourse.tile as tile

    F32 = mybir.dt.float32
    BF = mybir.dt.bfloat16
    Add = mybir.AluOpType.add
    Relu = mybir.ActivationFunctionType.Relu

    nc = bacc.Bacc("TRN2", target_bir_lowering=False, debug=False)

    eap_d = nc.dram_tensor("eap", [D, ct], BF, kind="ExternalInput")
    xt_d = nc.dram_tensor("xt", [D, NPC], BF, kind="ExternalInput")
    ugt_d = nc.dram_tensor("ugt", [DG, NPC], BF, kind="ExternalInput")
    w1x_d = nc.dram_tensor("w1x", [D, H], BF, kind="ExternalInput")
    w1a_d = nc.dram_tensor("w1a", [D, H], BF, kind="ExternalInput")
    w1u_d = nc.dram_tensor("w1u", [DG, H], BF, kind="ExternalInput")
    b1t_d = nc.dram_tensor("b1t", [D, 2], F32, kind="ExternalInput")
    w2a_d = nc.dram_tensor("w2a", [D, D], BF, kind="ExternalInput")
    w2b_d = nc.dram_tensor("w2b", [D, D], BF, kind="ExternalInput")
    b2r_d = nc.dram_tensor("b2r", [1, D], BF, kind="ExternalInput")
    ones_d = nc.dram_tensor("ones", [1, NB], BF, kind="ExternalInput")
    ident_d = nc.dram_tensor("ident", [D, D], BF, kind="ExternalInput")
    out_d = nc.dram_tensor("out", [D, NPC], BF, kind="ExternalOutput")

    with tile.TileContext(nc) as tc, ExitStack() as ctx:
        persist = ctx.enter_context(tc.tile_pool(name="persist", bufs=1))
        agg_pool = ctx.enter_context(tc.tile_pool(name="agg", bufs=1))
        outc_pool = ctx.enter_context(tc.tile_pool(name="outc", bufs=NCHUNK))
        hs_pool = ctx.enter_context(tc.tile_pool(name="hs", bufs=4))
        h_psum = ctx.enter_context(tc.tile_pool(name="hps", bufs=4, space="PSUM"))
        o2_psum = ctx.enter_context(tc.tile_pool(name="o2ps", bufs=2, space="PSUM"))

        def pload(dram, shape, dtype):
            t = persist.tile(shape, dtype, tag=dram.name)
            nc.scalar.dma_start(t[:], dram.ap())
            return t

        w1x_t = pload(w1x_d, [D, H], BF)
        w1a_t = pload(w1a_d, [D, H], BF)
        w1u_t = pload(w1u_d, [DG, H], BF)
        b1t_t = pload(b1t_d, [D, 2], F32)
        w2a_t = pload(w2a_d, [D, D], BF)
        w2b_t = pload(w2b_d, [D, D], BF)
        b2r_t = pload(b2r_d, [1, D], BF)
        ones_t = pload(ones_d, [1, NB], BF)
        ident_t = pload(ident_d, [D, D], BF)
        xt_t = pload(xt_d, [D, NPC], BF)
        ugt_t = pload(ugt_d, [DG, NPC], BF)

        # --- scatter: big HWDGE slab loads + DVE add trees per chunk -------
        # Full passes load 4-at-a-time (one 3.2 MB DMA); a pair-add tree
        # reduces each load with only ONE serialized add onto agg, so the
        # per-chunk dependency chain is ~4x shorter than a naive chain.
        RUN = 4
        slab_pool = ctx.enter_context(tc.tile_pool(name="slab", bufs=4))
        tmp_pool = ctx.enter_context(tc.tile_pool(name="tmp", bufs=6))
        agg_tiles = {}
        for c in range(NCHUNK):
            m = chunk_meta[c]
            agg = agg_pool.tile([D, CW], BF, tag=f"agg{c}")
            agg_tiles[c] = agg
            nc.sync.dma_start(agg[:], eap_d.ap()[:, m["base"]:m["base"] + CW])
            fb = m["base"] + CW
            i = 0
            nfull = m["fulls"]
            while i < nfull:
                n = min(RUN, nfull - i)
                t = slab_pool.tile([D, RUN * CW], BF, tag="slab")
                nc.sync.dma_start(
                    t[:, 0:n * CW],
                    eap_d.ap()[:, fb + i * CW:fb + (i + n) * CW])
                if n == 4:
                    a = tmp_pool.tile([D, CW], BF, tag="tmp")
                    b = tmp_pool.tile([D, CW], BF, tag="tmp")
                    nc.vector.tensor_add(a[:], t[:, 0:CW], t[:, CW:2 * CW])
                    nc.vector.tensor_add(b[:], t[:, 2 * CW:3 * CW],
                                         t[:, 3 * CW:4 * CW])
                    nc.vector.tensor_add(a[:], a[:], b[:])
                    nc.vector.tensor_add(agg[:], agg[:], a[:])
                elif n == 3:
                    a = tmp_pool.tile([D, CW], BF, tag="tmp")
                    nc.vector.tensor_add(a[:], t[:, 0:CW], t[:, CW:2 * CW])
                    nc.vector.tensor_add(a[:], a[:], t[:, 2 * CW:3 * CW])
                    nc.vector.tensor_add(agg[:], agg[:], a[:])
                elif n == 2:
                    a = tmp_pool.tile([D, CW], BF, tag="tmp")
                    nc.vector.tensor_add(a[:], t[:, 0:CW], t[:, CW:2 * CW])
                    nc.vector.tensor_add(agg[:], agg[:], a[:])
                else:
                    nc.vector.tensor_add(agg[:], agg[:], t[:, 0:CW])
                i += n
            # partial passes: pack consecutive segs into shared loads
            parts = m["partials"]
            i = 0
            while i < len(parts):
                n = 0
                tot = 0
                while (i + n < len(parts) and tot + parts[i + n][0] <= RUN * CW):
                    tot += parts[i + n][0]
                    n += 1
                n = max(n, 1)
                tot = sum(wp for wp, _ in parts[i:i + n])
                t = slab_pool.tile([D, RUN * CW], BF, tag="slab")
                base_off = parts[i][1]
                nc.sync.dma_start(t[:, 0:tot],
                                  eap_d.ap()[:, base_off:base_off + tot])
                o = 0
                for wp, _ in parts[i:i + n]:
                    nc.vector.tensor_add(agg[:, 0:wp], agg[:, 0:wp],
                                         t[:, o:o + wp])
                    o += wp
                i += n

        # --- MLP over 392-node groups, chunk by chunk ----------------------
        for c in range(NCHUNK):
            agg = agg_tiles[c]
            outc = outc_pool.tile([D, CW], BF, tag="outc")
            for q in range(CW // NB):
                off = q * NB
                gs = c * CW + off
                hs = []
                for ht in range(2):
                    hp = h_psum.tile([D, NB], F32, tag="hp")
                    hsl = slice(ht * D, (ht + 1) * D)
                    nc.tensor.matmul(hp[:], w1x_t[:, hsl], xt_t[:, gs:gs + NB],
                                     start=True, stop=False)
                    nc.tensor.matmul(hp[:], w1a_t[:, hsl], agg[:, off:off + NB],
                                     start=False, stop=False)
                    nc.tensor.matmul(hp[:], w1u_t[:, hsl], ugt_t[:, gs:gs + NB],
                                     start=False, stop=True)
                    hsb = hs_pool.tile([D, NB], BF, tag="hs")
                    nc.scalar.activation(hsb[:], hp[:], Relu,
                                         bias=b1t_t[:, ht:ht + 1])
                    hs.append(hsb)
                o2 = o2_psum.tile([D, NB], F32, tag="o2")
                nc.tensor.matmul(o2[:], w2a_t[:], hs[0][:], start=True, stop=False)
                nc.tensor.matmul(o2[:], w2b_t[:], hs[1][:], start=False, stop=False)
                nc.tensor.matmul(o2[:], ident_t[:], xt_t[:, gs:gs + NB],
                                 start=False, stop=False)
                nc.tensor.matmul(o2[:], b2r_t[:], ones_t[:], start=False, stop=True)
                nc.scalar.copy(outc[:, off:off + NB], o2[:])
            nc.scalar.dma_start(out_d.ap()[:, c * CW:(c + 1) * CW], outc[:])

    nc.compile()
    return nc


def kernel(**inputs) -> np.ndarray:
    in_maps, chunk_meta, ct, nodes_by_core = _shard_inputs(
        inputs["x"], inputs["edge_index"], inputs["edge_attr"], inputs["u"],
        inputs["batch"], inputs["W1"], inputs["b1"], inputs["W2"], inputs["b2"],
    )
    nc = _build_program(chunk_meta, ct)

    from concourse.bass_utils import run_bass_kernel_spmd

    res = run_bass_kernel_spmd(nc, in_maps, list(range(NCORES)))
    _PROFILE_RESULTS[0] = res
    full = np.empty((N_PAD, D), dtype=np.float32)
    for c in range(NCORES):
        full[nodes_by_core[c]] = res.results[c]["out"].astype(np.float32).T
    return np.ascontiguousarray(full[:N_NODES])
